# revision 20
# baseline (speedup 1.0000x reference)
"""Single-head full attention (B=4, S=4096, D=512) on 8 TRN2 NeuronCores.

Sharding: core c handles batch b = c//2, query half h = c%2 (2048 queries).

Key algebraic fold: scores = (x_q Wq^T)(x Wk^T)^T / sqrt(D)
                           = x_q @ M @ x^T,   M = Wq^T Wk / sqrt(D)  (host).
So K is never materialized: x^T itself (resident in SBUF, fp16) is the
stationary operand of the scores matmul, and T = x_q @ M replaces Q.
Per-query additive terms drop out of softmax (row-shift invariance); with
biases the per-key additive beta[j] = (bq Wk/sqrt(D))x[j]^T is applied as a
multiplier exp(beta) on the exp'd scores (the bq.bk constant cancels).

Key-permutation trick: the host sends x^T with the core's OWN query half in
columns 0:SQ (halves swapped for odd cores). Softmax and P@V are invariant
to key order (V is projected from the same permuted x^T), so one SPMD
program serves all cores and the separate x_q^T upload disappears: the
T-projection just reads columns 0:SQ of the resident x^T.

Device layouts (per core, fp16 operands, fp32 accumulate):
  xt_sb [128, 4, 4096]: x^T (query-half-first), partition p + tile t -> d'
  tt_sb [128, 4, 2048]: T^T = (x_q @ M)^T
  v_sb  [128, 32, 512]: V natural, partition p + block jb -> j = jb*128+p
Scores are computed transposed (S^T[j, q]) so exp(S^T) blocks serve directly
as the stationary operand of the P@V matmul, producing O in natural [q, d]
orientation. Softmax denominators come from an N=2 ones-matmul sharing the
same stationary tile (the LDWEIGHTS rides on the P@V matmul's), so the
denominator is written twice; the four per-subblock denominator groups share
one PSUM bank, zeroed once, all groups accumulate with start=False.
No max-subtraction: scores are O(1), softmax is shift-invariant.

P@V runs in fp8e4 DoubleRow (2 MACs/cell/cycle, measured 2.0x over fp16):
exp writes fp8e4 directly (bias=-2 inside the exp keeps max ~53 under the
TRN e4 saturation at 240; the e^-2 factor cancels in num/den), V is stored
fp8e4 from the projection PSUM, and key blocks are processed in pairs so
the stationary is [128, 2, 128] (contraction 256). P@V of pair p issues
after the scores of pair p+2 (lag-2 software pipeline) so the in-order PE
stream never waits on ACT's exp. A post-compile pass deletes the
InstLdweights that the denominator ones-matmul would use to reload the
P@V matmul's stationary (the tile scheduler pairs every matmul with its
own load; the duplicate is pure overhead). The output is stored fp16 and
gathered to fp32 on host. End-to-end rel err vs the fp32 reference is
1.899e-2 (deterministic seed-0 inputs), under the 2e-2 gate;
scores/projections stay fp16.

Startup: m is loaded in 4 column chunks (the first T-proj group needs only
chunk 0) and x^T in 8 column chunks, so the first matmul starts ~2us in.
Tail: output tiles are scaled in halves (DVE + ACT in parallel) and stored
via both HWDGE queues (SP + ACT) to shorten the post-matmul drain.
"""
import math
import numpy as np

B, S, D = 4, 4096, 512
P = 128
SQ = S // 2          # queries per core
NCORES = 8
QTILE = 512          # query columns per score/PV pass

last_results = None  # BassKernelResults of the most recent run (for test.py)

# bench-only ablation switches (test harness sets these; all-off = real kernel)
VARIANT = {}

_nc_cache = {}


def _build_nc(has_bias, has_mask, reps=1):
    import concourse.bacc as bacc
    import concourse.tile as tile
    from concourse import mybir
    from contextlib import ExitStack

    f32 = mybir.dt.float32
    f16 = mybir.dt.float16
    f8 = mybir.dt.float8e4
    DR = mybir.MatmulPerfMode.DoubleRow
    Exp = mybir.ActivationFunctionType.Exp
    Copy = mybir.ActivationFunctionType.Copy
    fp8_pv = not (has_bias or has_mask)
    EXPB = -2.0 if fp8_pv else 0.0

    nc = bacc.Bacc("TRN2", target_bir_lowering=False, debug=False)
    xT = nc.declare_dram_parameter("xT", [D, S], f16, False)
    mT = nc.declare_dram_parameter("mT", [D, D], f16, False)
    wvT = nc.declare_dram_parameter("wvT", [D, D], f16, False)
    if has_bias:
        wtl = nc.declare_dram_parameter("wtl", [P, D // P], f16, False)
        bvr = nc.declare_dram_parameter("bvr", [P, D], f32, False)
    if has_mask:
        maskf = nc.declare_dram_parameter("maskf", [P, S // P], f32, False)
    # fp16 output store: halves the output DMA; host gathers back to fp32.
    y = nc.declare_dram_parameter("y", [SQ, D], f16 if fp8_pv else f32, True)

    ET = D // P          # 4 d'-tiles
    NJB = S // P         # 32 key blocks
    NQT = SQ // QTILE    # 4 query tiles
    NQS = QTILE // P     # 4 query subblocks per tile
    HD = D // 2

    with tile.TileContext(nc) as tc, ExitStack() as ctx:
        wpool = ctx.enter_context(tc.tile_pool(name="wpool", bufs=1))
        big = ctx.enter_context(tc.tile_pool(name="big", bufs=1))
        expp = ctx.enter_context(tc.tile_pool(name="expp", bufs=6))
        outp = ctx.enter_context(tc.tile_pool(name="outp", bufs=4))
        smallp = ctx.enter_context(tc.tile_pool(name="smallp", bufs=3))
        # PSUM: shared [128,512] accumulate tag (projections + scores) keeps
        # every phase inside 8 banks: 3 (mm512) + 4 (po) + 1 (sums).
        psum_mm = ctx.enter_context(tc.tile_pool(name="psum_mm", bufs=3, space="PSUM"))
        psum_o = ctx.enter_context(tc.tile_pool(name="psum_o", bufs=1, space="PSUM"))
        psum_sum = ctx.enter_context(tc.tile_pool(name="psum_sum", bufs=1, space="PSUM"))

        m_sb = wpool.tile([P, ET, D], f16)
        wv_sb = wpool.tile([P, ET, D], f16)
        # m+wv ride the ACT HWDGE queue (idle until the first exp) so they
        # never sit in front of the x^T chunks on the SP queue; m arrives in
        # two column chunks so the first T-proj group (me=0,1) unlocks early.
        for mh in range(4):
            nc.scalar.dma_start(
                out=m_sb[:, :, mh * P:(mh + 1) * P],
                in_=mT[:, mh * P:(mh + 1) * P]
                .rearrange("(t p) e -> p t e", p=P))
        nc.scalar.dma_start(out=wv_sb,
                            in_=wvT[:, :].rearrange("(t p) e -> p t e", p=P))
        if fp8_pv:
            ones_sb = wpool.tile([P, 2, 2], f8)
            expb_sb = wpool.tile([P, 1], f32)
            nc.vector.memset(expb_sb, EXPB)
        else:
            ones_sb = wpool.tile([P, 2], f16)
        nc.vector.memset(ones_sb, 1.0)
        if has_bias:
            wtl_sb = wpool.tile([P, D // P], f16)
            bv_sb = wpool.tile([P, D], f32)
            nc.sync.dma_start(out=wtl_sb, in_=wtl[:, :])
            nc.sync.dma_start(out=bv_sb, in_=bvr[:, :])
        if has_mask:
            mask_sb = wpool.tile([P, S // P], f32)
            nc.sync.dma_start(out=mask_sb, in_=maskf[:, :])

        xt_sb = big.tile([P, ET, S], f16)
        tt_sb = big.tile([P, ET, SQ], f16)
        v_sb = big.tile([P, NJB, D], f8 if fp8_pv else f16)

        xT_r = xT[:, :].rearrange("(t p) s -> p t s", p=P)

        def body(rep):
            # resident x^T load, chunked so consumers unlock early; columns
            # 0:SQ are this core's query half (host pre-permuted). Chunk 0
            # is split in two so the very first matmul group starts after
            # only 256 columns have landed.
            HQ = QTILE // 2
            nc.sync.dma_start(out=xt_sb[:, :, 0:P], in_=xT_r[:, :, 0:P])
            nc.sync.dma_start(out=xt_sb[:, :, P:HQ], in_=xT_r[:, :, P:HQ])
            nc.sync.dma_start(out=xt_sb[:, :, HQ:QTILE], in_=xT_r[:, :, HQ:QTILE])
            for c in range(1, S // QTILE):
                eng = nc.scalar if (VARIANT.get("xt2q") and c >= 4) else nc.sync
                eng.dma_start(
                    out=xt_sb[:, :, c * QTILE:(c + 1) * QTILE],
                    in_=xT_r[:, :, c * QTILE:(c + 1) * QTILE])

            # T^T projection: M-stationary, x_q^T-moving (= x^T cols 0:SQ);
            # the c=0 tile runs as two half-width groups matching the split
            # chunk-0 load.
            for c in range(0 if VARIANT.get("no_tproj") else SQ // QTILE):
                for me in range(ET):
                    pq = psum_mm.tile([P, QTILE], f32, tag="mm512",
                                      name=f"pq_{rep}_{c}_{me}")
                    if c == 0:
                        for hf in range(4):
                            sl = slice(hf * P, (hf + 1) * P)
                            for t in range(ET):
                                nc.tensor.matmul(
                                    pq[:, sl],
                                    lhsT=m_sb[:, t, me * P:(me + 1) * P],
                                    rhs=xt_sb[:, t, sl],
                                    start=(t == 0), stop=(t == ET - 1))
                    else:
                        for t in range(ET):
                            nc.tensor.matmul(
                                pq,
                                lhsT=m_sb[:, t, me * P:(me + 1) * P],
                                rhs=xt_sb[:, t, c * QTILE:(c + 1) * QTILE],
                                start=(t == 0), stop=(t == ET - 1))
                    if (c * ET + me) % 2:
                        nc.scalar.copy(
                            out=tt_sb[:, me, c * QTILE:(c + 1) * QTILE], in_=pq)
                    else:
                        nc.vector.tensor_copy(
                            out=tt_sb[:, me, c * QTILE:(c + 1) * QTILE], in_=pq)

            # V projection: x^T-stationary, Wv^T-moving
            for sb_i in range(0 if VARIANT.get("no_vproj") else NJB):
                pv = psum_mm.tile([P, D], f32, tag="mm512", name=f"pv_{rep}_{sb_i}")
                for t in range(ET):
                    nc.tensor.matmul(
                        pv,
                        lhsT=xt_sb[:, t, sb_i * P:(sb_i + 1) * P],
                        rhs=wv_sb[:, t, :],
                        start=(t == 0), stop=(t == ET - 1))
                nc.vector.tensor_copy(out=v_sb[:, sb_i, :], in_=pv)

            # per-key bias multiplier exp(beta[j]) (only when biases present)
            if has_bias:
                bmul_sb = smallp.tile([P, NJB], f32, tag="bmul", name=f"bm_{rep}")
                for jb in range(NJB):
                    # rides the sums tag/bank: a dedicated bsum tag would need
                    # a 9th PSUM bank
                    pb = psum_sum.tile([P, 2 * NQS], f32, tag="sums",
                                       name=f"pb_{rep}_{jb}")
                    for t in range(ET):
                        nc.tensor.matmul(
                            pb[:, 0:2],
                            lhsT=xt_sb[:, t, jb * P:(jb + 1) * P],
                            rhs=wtl_sb[:, t:t + 1].to_broadcast([P, 2]),
                            start=(t == 0), stop=(t == ET - 1))
                    nc.scalar.activation(out=bmul_sb[:, jb:jb + 1], in_=pb[:, 0:1],
                                         func=Exp, scale=1.0)

            # attention
            for qt in range(0 if VARIANT.get("no_attn") else NQT):
                po = [psum_o.tile([P, D], f32, tag=f"po{qs}", name=f"po_{rep}_{qt}_{qs}")
                      for qs in range(NQS)]
                psums = psum_sum.tile([P, 2 * NQS], f32, tag="sums",
                                      name=f"sums_{rep}_{qt}")
                nc.vector.memset(psums, 0.0)
                if fp8_pv:
                    # key blocks in pairs: exp(S^T) of both blocks lands in a
                    # [P, 2, QTILE] fp8 tile whose qs-slices are the DoubleRow
                    # stationaries (contraction 256) of the P@V matmul.
                    # P@V of pair p issues AFTER the scores of pair p+1 so the
                    # in-order PE queue never waits on ACT's exp of pair p.
                    def emit_pv(pexp, jbp):
                        last = jbp == NJB // 2 - 1
                        qs_order = range(NQS)
                        if VARIANT.get("ones_after"):
                            for qs in qs_order:
                                nc.tensor.matmul(
                                    po[qs],
                                    lhsT=pexp[:, :, qs * P:(qs + 1) * P],
                                    rhs=v_sb[:, 2 * jbp:2 * jbp + 2, :],
                                    start=(jbp == 0), stop=last,
                                    perf_mode=DR)
                            if not VARIANT.get("no_ones"):
                                for qs in qs_order:
                                    nc.tensor.matmul(
                                        psums[:, 2 * qs:2 * qs + 2],
                                        lhsT=pexp[:, :, qs * P:(qs + 1) * P],
                                        rhs=ones_sb,
                                        start=False, stop=last,
                                        perf_mode=DR,
                                        skip_group_check=True)
                            return
                        for qs in qs_order:
                            nc.tensor.matmul(
                                po[qs],
                                lhsT=pexp[:, :, qs * P:(qs + 1) * P],
                                rhs=v_sb[:, 2 * jbp:2 * jbp + 2, :],
                                start=(jbp == 0), stop=last,
                                perf_mode=DR)
                            if not VARIANT.get("no_ones"):
                                nc.tensor.matmul(
                                    psums[:, 2 * qs:2 * qs + 2],
                                    lhsT=pexp[:, :, qs * P:(qs + 1) * P],
                                    rhs=ones_sb,
                                    start=False, stop=last,
                                    perf_mode=DR,
                                    skip_group_check=True)

                    cpexp = None
                    if VARIANT.get("no_exp"):
                        cpexp = expp.tile([P, 2, QTILE], f8, tag="cpexp",
                                          name=f"cpe_{rep}_{qt}")
                        nc.vector.memset(cpexp, 0.25)
                    pending = []
                    for jbp in range(NJB // 2):
                        pexp = expp.tile([P, 2, QTILE], f8, tag="pexp",
                                         name=f"pe_{rep}_{qt}_{jbp}")
                        for jj in range(2):
                            jb = 2 * jbp + jj
                            ps_t = psum_mm.tile([P, QTILE], f32, tag="mm512",
                                                name=f"ps_{rep}_{qt}_{jb}")
                            if not VARIANT.get("no_scores"):
                                for t in range(ET):
                                    nc.tensor.matmul(
                                        ps_t,
                                        lhsT=xt_sb[:, t, jb * P:(jb + 1) * P],
                                        rhs=tt_sb[:, t, qt * QTILE:(qt + 1) * QTILE],
                                        start=(t == 0), stop=(t == ET - 1))
                            if not VARIANT.get("no_exp"):
                                src = ps_t
                                if VARIANT.get("exp_via_sbuf"):
                                    sc = expp.tile([P, QTILE], f32, tag="scop",
                                                   name=f"sc_{rep}_{qt}_{jb}")
                                    nc.vector.tensor_copy(out=sc, in_=ps_t)
                                    src = sc
                                nc.scalar.activation(out=pexp[:, jj, 0:QTILE // 2],
                                                     in_=src[:, 0:QTILE // 2],
                                                     func=Exp, scale=1.0, bias=expb_sb)
                                nc.scalar.activation(out=pexp[:, jj, QTILE // 2:QTILE],
                                                     in_=src[:, QTILE // 2:QTILE],
                                                     func=Exp, scale=1.0, bias=expb_sb)
                        use = cpexp if VARIANT.get("no_exp") else pexp
                        if VARIANT.get("no_pv"):
                            continue
                        if VARIANT.get("no_pipeline"):
                            emit_pv(use, jbp)
                        else:
                            pending.append((use, jbp))
                            # lag-2: exp of pair p gets two pairs of PE time
                            # before its P@V issues, so the in-order PE queue
                            # never blocks on ACT.
                            if len(pending) > 2:
                                emit_pv(*pending.pop(0))
                    for item in pending:
                        emit_pv(*item)
                else:
                    for jb in range(NJB):
                        ps_t = psum_mm.tile([P, QTILE], f32, tag="mm512",
                                            name=f"ps_{rep}_{qt}_{jb}")
                        for t in range(ET):
                            nc.tensor.matmul(
                                ps_t,
                                lhsT=xt_sb[:, t, jb * P:(jb + 1) * P],
                                rhs=tt_sb[:, t, qt * QTILE:(qt + 1) * QTILE],
                                start=(t == 0), stop=(t == ET - 1))
                        pexp = expp.tile([P, QTILE], f16, tag="pexp",
                                         name=f"pe_{rep}_{qt}_{jb}")
                        # exp in two halves: the first P@V stationary (qs=0)
                        # unlocks ~300ns earlier, closing the exp->PV latency
                        # gap that otherwise stalls PE every other key block.
                        nc.scalar.activation(out=pexp[:, 0:QTILE // 2],
                                             in_=ps_t[:, 0:QTILE // 2],
                                             func=Exp, scale=1.0)
                        nc.scalar.activation(out=pexp[:, QTILE // 2:QTILE],
                                             in_=ps_t[:, QTILE // 2:QTILE],
                                             func=Exp, scale=1.0)
                        if has_bias:
                            nc.vector.tensor_scalar_mul(pexp, pexp, bmul_sb[:, jb:jb + 1])
                        if has_mask:
                            nc.vector.tensor_scalar_mul(pexp, pexp, mask_sb[:, jb:jb + 1])
                        for qs in range(NQS):
                            nc.tensor.matmul(
                                po[qs],
                                lhsT=pexp[:, qs * P:(qs + 1) * P],
                                rhs=v_sb[:, jb, :],
                                start=(jb == 0), stop=(jb == NJB - 1))
                            nc.tensor.matmul(
                                psums[:, 2 * qs:2 * qs + 2],
                                lhsT=pexp[:, qs * P:(qs + 1) * P],
                                rhs=ones_sb,
                                start=False, stop=(jb == NJB - 1),
                                skip_group_check=True)
                recip = smallp.tile([P, 2 * NQS], f32, tag="recip", name=f"rc_{rep}_{qt}")
                if fp8_pv:
                    # per-qs reciprocal: each drain starts as soon as its own
                    # denominator group stops instead of after all four.
                    for qs in range(NQS):
                        nc.vector.reciprocal(out=recip[:, 2 * qs:2 * qs + 2],
                                             in_=psums[:, 2 * qs:2 * qs + 2])
                else:
                    nc.vector.reciprocal(out=recip, in_=psums)
                if VARIANT.get("no_pv"):
                    continue
                odt = f16 if fp8_pv else f32
                if has_bias:
                    for qs in range(NQS):
                        r0 = (qt * NQS + qs) * P
                        o_sb = outp.tile([P, D], f32, tag="osb",
                                         name=f"o_{rep}_{qt}_{qs}")
                        nc.vector.tensor_scalar_mul(o_sb, po[qs],
                                                    recip[:, 2 * qs:2 * qs + 1])
                        nc.vector.tensor_add(out=o_sb, in0=o_sb, in1=bv_sb)
                        nc.sync.dma_start(out=y[r0:r0 + P, :], in_=o_sb)
                else:
                    # pairwise-batched stores (one dma_start per 2 tiles: the
                    # ~1.2us descriptor-issue cost dominates the transfer).
                    # On the final qt, each tile is scaled and stored on its
                    # own (alternating DVE/ACT scales and SP/ACT queues) so
                    # the first transfer starts as soon as its scale is done.
                    last = qt == NQT - 1
                    if last:
                        # final drain: two paired stores on parallel queues
                        # (descriptor issue ~1.2us dominates the transfer);
                        # within a pair the scales run on DVE and ACT
                        # concurrently.
                        for half in range(2):
                            o2 = outp.tile([P, 2, D], odt, tag=f"o2l_{half}",
                                           name=f"o2l_{rep}_{qt}_{half}")
                            for k in range(2):
                                qs = 2 * half + k
                                if k:
                                    nc.scalar.activation(
                                        out=o2[:, k, :], in_=po[qs], func=Copy,
                                        scale=recip[:, 2 * qs:2 * qs + 1])
                                else:
                                    nc.vector.tensor_scalar_mul(
                                        o2[:, k, :], po[qs],
                                        recip[:, 2 * qs:2 * qs + 1])
                            r0 = (qt * NQS + 2 * half) * P
                            dma_eng = nc.sync if half == 0 else nc.scalar
                            dma_eng.dma_start(
                                out=y[r0:r0 + 2 * P, :].rearrange(
                                    "(k p) d -> p k d", p=P),
                                in_=o2)
                    else:
                        for half in range(2):
                            o2 = outp.tile([P, 2, D], odt, tag=f"o2_{half}",
                                           name=f"o2_{rep}_{qt}_{half}")
                            for k in range(2):
                                qs = 2 * half + k
                                nc.vector.tensor_scalar_mul(
                                    o2[:, k, :], po[qs],
                                    recip[:, 2 * qs:2 * qs + 1])
                            r0 = (qt * NQS + 2 * half) * P
                            nc.sync.dma_start(
                                out=y[r0:r0 + 2 * P, :].rearrange(
                                    "(k p) d -> p k d", p=P),
                                in_=o2)

        if reps == 1:
            body(0)
        else:
            # bench-only loop; hint the big-body engines so the back-edge
            # branch prefetches its IRAM block instead of stalling ~4us
            with tc.For_i(0, reps, 1,
                          hint_engines=(mybir.EngineType.PE,
                                        mybir.EngineType.Activation,
                                        mybir.EngineType.DVE,
                                        mybir.EngineType.SP)):
                body(0)
    nc.compile()
    _dedup_ldweights(nc, mybir)
    return nc


def _dedup_ldweights(nc, mybir):
    """Drop an InstLdweights that reloads the stationary the PE already holds.

    The tile scheduler pairs every InstMatmult with its own InstLdweights; the
    denominator ones-matmul shares the P@V matmul's stationary (same AP, same
    perf mode), so its reload is pure overhead (~256-row DoubleRow load that
    the reorder window cannot always hide). Only sync-free duplicates whose
    PE-stream gap contains nothing but InstMatmult are removed.
    """
    PE = mybir.EngineType.PE
    for bb in nc.m.functions[0].blocks:
        insts = bb.instructions
        prev_key = None
        to_del = []
        for k, inst in enumerate(insts):
            if getattr(inst, "engine", None) != PE:
                continue
            nm = type(inst).__name__
            if nm == "InstLdweights":
                key = (str(inst.ins[0]), str(inst.perf_mode),
                       str(inst.is_transpose), str(inst.tile_position))
                if (key == prev_key and not inst.has_wait()
                        and not inst.has_update()):
                    to_del.append(k)
                else:
                    prev_key = key
            elif nm != "InstMatmult":
                prev_key = None  # anything else on PE may clobber the array
        for k in reversed(to_del):
            del insts[k]


def _prepare(x, mask, Wq, bq, Wk, bk, Wv, bv):
    """Build (or fetch cached) device program + per-core input maps."""
    x = np.asarray(x, dtype=np.float32)
    mask = np.asarray(mask)
    Wq = np.asarray(Wq, dtype=np.float32)
    Wk = np.asarray(Wk, dtype=np.float32)
    Wv = np.asarray(Wv, dtype=np.float32)
    bq = np.asarray(bq, dtype=np.float32)
    bk = np.asarray(bk, dtype=np.float32)
    bv = np.asarray(bv, dtype=np.float32)
    has_bias = bool(np.any(bq) or np.any(bk) or np.any(bv))
    has_mask = bool(np.any(mask))

    key = (has_bias, has_mask)
    if key not in _nc_cache:
        _nc_cache[key] = _build_nc(has_bias, has_mask)
    nc = _nc_cache[key]

    inv_sqrt_d = 1.0 / math.sqrt(D)
    M = (Wq.T.astype(np.float64) @ Wk.astype(np.float64)) * inv_sqrt_d
    mT_h = np.ascontiguousarray(M.astype(np.float32).astype(np.float16))
    wvT_h = np.ascontiguousarray(Wv.T.astype(np.float16))

    in_maps = []
    for c in range(NCORES):
        b, h = divmod(c, 2)
        xT_b = x[b].T.astype(np.float16)
        if h:
            # key order is free (softmax + P@V are permutation-invariant over
            # keys); put this core's query half in columns 0:SQ.
            xT_b = np.concatenate([xT_b[:, SQ:], xT_b[:, :SQ]], axis=1)
        m = {
            "xT": np.ascontiguousarray(xT_b),
            "mT": mT_h, "wvT": wvT_h,
        }
        if has_bias:
            # per-key additive beta[j] = (bq Wk/sqrt(D)).x[j]; the bq.bk
            # constant shifts all keys equally and cancels in softmax.
            wt = (bq @ Wk) * inv_sqrt_d              # [D]
            m["wtl"] = np.ascontiguousarray(
                wt.reshape(D // P, P).T.astype(np.float16))
            m["bvr"] = np.ascontiguousarray(np.broadcast_to(bv, (P, D))).copy()
        if has_mask:
            keep = 1.0 - mask[b].astype(np.float32)
            if h:
                keep = np.concatenate([keep[SQ:], keep[:SQ]])
            m["maskf"] = np.ascontiguousarray(keep.reshape(S // P, P).T)
        in_maps.append(m)
    return nc, in_maps


def _gather(res):
    out = np.empty((B, S, D), dtype=np.float32)
    for c in range(NCORES):
        b, h = divmod(c, 2)
        out[b, h * SQ:(h + 1) * SQ, :] = res.results[c]["y"]
    return out


def kernel(x, mask, Wq, bq, Wk, bk, Wv, bv):
    global last_results
    from concourse.bass_utils import run_bass_kernel_spmd

    nc, in_maps = _prepare(x, mask, Wq, bq, Wk, bk, Wv, bv)
    res = run_bass_kernel_spmd(nc, in_maps, core_ids=list(range(NCORES)))
    last_results = res
    return _gather(res)

